# revision 3
# baseline (speedup 1.0000x reference)
"""Trainium2 Bass kernel for nn_Convolution_49125835932146.

The whole module (kernel synthesis + self-interaction + conv3d) reduces to a
single 64->64 channel 5x5x5 conv3d: the per-voxel self-interaction linear is
folded into the center tap of the (host-synthesized, tiny) conv kernel.

Distribution: 8 shards = batch(4) x z-half(2), one per NeuronCore. Each core
computes a (64, 16, 32, 32) output shard.

Per-core compute: PSUM tile = 128 partitions (64 out-ch x 2 adjacent y rows)
x 512 free (16 z x 32 x). Contraction K=128 = 64 in-ch x 2 adjacent y input
slices (SBUF holds x and x-shifted-by-one-y-row in partitions 0-63 / 64-127).
75 accumulating float32r matmuls per y-row-pair (5 dz x 5 dx x 3 slice-pairs)
cover all 125 taps; 10 of every 12 weight blocks are useful (83% PE util).
"""

import math

import numpy as np

# ---------------------------------------------------------------------------
# Host-side weight synthesis (mirrors the reference's kernel-synthesis stage).
# ---------------------------------------------------------------------------

SIZE = 5
MUL = 16


def _real_sh(v):
    x = v[..., 0]; y = v[..., 1]; z = v[..., 2]
    r2 = x * x + y * y + z * z
    s3 = math.sqrt(3.0); s5 = math.sqrt(5.0); s15 = math.sqrt(15.0)
    s7 = math.sqrt(7.0); s105 = math.sqrt(105.0)
    c21 = 0.5 * math.sqrt(10.5)
    c35 = 0.5 * math.sqrt(17.5)
    out = [
        np.ones_like(x),
        s3 * y, s3 * z, s3 * x,
        s15 * x * y, s15 * y * z, 0.5 * s5 * (3 * z * z - r2),
        s15 * x * z, 0.5 * s15 * (x * x - y * y),
        c35 * y * (3 * x * x - y * y), s105 * x * y * z,
        c21 * y * (5 * z * z - r2), 0.5 * s7 * z * (5 * z * z - 3 * r2),
        c21 * x * (5 * z * z - r2), 0.5 * s105 * z * (x * x - y * y),
        c35 * x * (x * x - 3 * y * y),
    ]
    return np.stack(out, axis=-1)


def _wigner3j(l1, l2, l3, rng):
    dims = (2 * l1 + 1, 2 * l2 + 1, 2 * l3 + 1)
    n = dims[0] * dims[1] * dims[2]
    rows = []
    for _ in range(4):
        A = rng.standard_normal((3, 3))
        Q, _ = np.linalg.qr(A)
        if np.linalg.det(Q) < 0:
            Q = -Q
        X = rng.standard_normal((64, 3))
        X /= np.linalg.norm(X, axis=1, keepdims=True)
        YA = _real_sh(X)
        YB = _real_sh(X @ Q.T)
        Ds = []
        for l in (l1, l2, l3):
            sl = slice(l * l, (l + 1) * (l + 1))
            D = np.linalg.lstsq(YA[:, sl], YB[:, sl], rcond=None)[0].T
            Ds.append(D)
        rows.append(np.kron(Ds[0], np.kron(Ds[1], Ds[2])) - np.eye(n))
    _, _, Vt = np.linalg.svd(np.concatenate(rows, 0))
    w = Vt[-1]
    w = w * np.sign(w[np.argmax(np.abs(w))])
    w = w / np.linalg.norm(w)
    return w.reshape(dims).astype(np.float32)


_W3J = None


def _w3j():
    global _W3J
    if _W3J is None:
        rng = np.random.default_rng(7)
        _W3J = {k: _wigner3j(*k, rng)
                for k in [(0, 0, 0), (0, 1, 1), (1, 0, 1), (1, 1, 0), (1, 2, 1)]}
    return _W3J


def _build_keff(w_lin0, w_lin1, tp_weight):
    """(64,64,5,5,5) conv kernel with the self-interaction folded into the
    center tap and the 0.1 conv scale applied: out = conv3d(x, keff, pad=2)."""
    size, mul = SIZE, MUL
    W3J = _w3j()
    r = np.linspace(-1.0, 1.0, size)
    lat = np.stack(np.meshgrid(r, r, r, indexing="ij"), axis=-1)
    d = np.sqrt(np.sum(lat * lat, axis=-1))
    v = lat / np.maximum(d, 1e-12)[..., None]
    sh = _real_sh(v)
    centers = np.linspace(0.0, 1.0, size)
    sigma = 1.0 / (size - 1)
    emb = np.exp(-np.square((d[..., None] - centers) / sigma)) / 1.12
    w = (emb @ tp_weight.astype(np.float64)) * np.cos(np.pi * d)[..., None] / size ** 1.5
    W = w.reshape(size, size, size, 5, mul, mul)
    f0 = math.sqrt(1.0 / (2 * mul))
    f1 = math.sqrt(3.0 / (3 * mul))

    def blk(C, ysl, Wp, f):
        t = f * np.einsum("abc,xyzb,xyzuw->xyzuawc", C, ysl, Wp)
        s = t.shape
        return t.reshape(size, size, size, s[3] * s[4], s[5] * s[6])

    y0 = sh[..., 0:1]; y1 = sh[..., 1:4]; y2 = sh[..., 4:9]
    b00 = blk(W3J[(0, 0, 0)], y0, W[..., 0, :, :], f0)
    b01 = blk(W3J[(0, 1, 1)], y1, W[..., 1, :, :], f1)
    b11a = blk(W3J[(1, 0, 1)], y0, W[..., 2, :, :], f1)
    b10 = blk(W3J[(1, 1, 0)], y1, W[..., 3, :, :], f0)
    b11b = blk(W3J[(1, 2, 1)], y2, W[..., 4, :, :], f1)
    k = np.concatenate([
        np.concatenate([b00, b01], axis=-1),
        np.concatenate([b10, b11a + b11b], axis=-1)], axis=-2)
    kernel = np.transpose(k, (4, 3, 0, 1, 2))  # oixyz

    inv = 1.0 / math.sqrt(mul)
    L = np.zeros((64, 64), np.float64)
    L[:16, :16] = inv * w_lin0.T
    L[16:, 16:] = inv * np.kron(w_lin1.T, np.eye(3))
    keff = 0.1 * kernel
    keff[:, :, 2, 2, 2] += L
    return keff.astype(np.float32)


def _build_lhst(keff):
    """(75, 128, 128) stationary matmul operands.
    m = (dz*5+dx)*3 + ui with u = 2*ui; W[m, i+64a, o+64b] = keff[o,i,dz,u+a-b,dx]."""
    W = np.zeros((75, 128, 128), np.float32)
    for dz in range(5):
        for dx in range(5):
            for ui, u in enumerate((0, 2, 4)):
                m = (dz * 5 + dx) * 3 + ui
                for a in (0, 1):
                    for b in (0, 1):
                        dy = u + a - b
                        if 0 <= dy < 5:
                            W[m, 64 * a:64 * a + 64, 64 * b:64 * b + 64] = \
                                keff[:, :, dz, dy, dx].T
    return W


# ---------------------------------------------------------------------------
# BIR post-processing: this container's walrus accepts at most ONE sync-wait
# per instruction; Tile emits multi-wait joins. Move extra waits onto
# injected same-engine NoOps placed immediately before the instruction.
# ---------------------------------------------------------------------------

def _split_multi_waits(bir_json: bytes) -> bytes:
    import orjson
    d = orjson.loads(bir_json)
    counter = [0]

    def fix_blocks(blocks):
        for b in blocks:
            insts = b.get("instructions", [])
            out = []
            for i in insts:
                si = i.get("sync_info")
                waits = (si or {}).get("on_wait") or []
                if len(waits) > 1:
                    for w in waits[:-1]:
                        counter[0] += 1
                        out.append({
                            "name": f"I-waitsplit-{counter[0]}",
                            "engine": i["engine"],
                            "opcode": "NoOp",
                            "ins": [],
                            "outs": [],
                            "debug": i.get("debug"),
                            "sync_info": {"on_wait": [w], "on_update": []},
                        })
                    si["on_wait"] = [waits[-1]]
                out.append(i)
            b["instructions"] = out
            fix_blocks(b.get("blocks", []))

    for fn in d["functions"]:
        fix_blocks(fn["blocks"])
    return orjson.dumps(d)


def _install_waitsplit():
    import concourse.bass_utils as bass_utils
    import concourse.bass2jax as bass2jax

    if getattr(bass_utils, "_waitsplit_installed", False):
        return
    orig = bass_utils.compile_bir_kernel

    def patched(bir_json, tmpdir, neff_name="file.neff"):
        return orig(_split_multi_waits(bytes(bir_json)), tmpdir, neff_name)

    bass_utils.compile_bir_kernel = patched
    bass_utils._waitsplit_installed = True
    if getattr(bass2jax, "compile_bir_kernel", None) is orig:
        bass2jax.compile_bir_kernel = patched


# ---------------------------------------------------------------------------
# Bass program (one per-core SPMD program, built once and cached).
# ---------------------------------------------------------------------------

_PROG = None

# SBUF x-tile geometry: 20 z-planes, 36 y-rows, 36 x-cols (all with halo).
ZT, YT, XT = 20, 36, 36
NVOX = ZT * YT * XT  # 25920 per partition
NW = 75


def _build_program():
    import concourse.bass as bass
    import concourse.mybir as mybir
    import concourse.tile as tile

    _install_waitsplit()

    nc = bass.Bass("TRN2", target_bir_lowering=False, debug=False)
    xs_in = nc.dram_tensor("xs", (128, NVOX), mybir.dt.float32r, kind="ExternalInput")
    wt_in = nc.dram_tensor("wt", (128, NW * 128), mybir.dt.float32r, kind="ExternalInput")
    out = nc.dram_tensor("out", (64, 16, 32, 32), mybir.dt.float32, kind="ExternalOutput")

    with tile.TileContext(nc) as tc:
        with tc.tile_pool(name="xpool", bufs=1) as xpool, \
             tc.tile_pool(name="wpool", bufs=1) as wpool, \
             tc.tile_pool(name="stage", bufs=4) as stpool, \
             tc.tile_pool(name="psum", bufs=4, space="PSUM") as pspool:
            xt = xpool.tile([128, ZT, YT, XT], mybir.dt.float32r)
            wt = wpool.tile([128, NW, 128], mybir.dt.float32r)
            # Load weights + x; split x DMA across partition slabs so several
            # HW queues run in parallel.
            nc.sync.dma_start(wt[:], wt_in.ap().rearrange("p (m c) -> p m c", m=NW))
            xs_ap = xs_in.ap().rearrange("p (z y x) -> p z y x", z=ZT, y=YT)
            xt_flat = xt
            for s in range(8):
                nc.sync.dma_start(xt_flat[16 * s:16 * (s + 1)],
                                  xs_ap[16 * s:16 * (s + 1)])
            for yp in range(16):
                y0 = 2 * yp
                ps = pspool.tile([128, 512], mybir.dt.float32)
                first = True
                for dz in range(5):
                    for dx in range(5):
                        for ui, u in enumerate((0, 2, 4)):
                            m = (dz * 5 + dx) * 3 + ui
                            rhs = xt[:, dz:dz + 16, y0 + u, dx:dx + 32]
                            last = (dz == 4 and dx == 4 and ui == 2)
                            nc.tensor.matmul(ps[:], wt[:, m, :], rhs,
                                             start=first, stop=last)
                            first = False
                st = stpool.tile([128, 512], mybir.dt.float32)
                nc.vector.tensor_copy(st[:], ps[:])
                # psum partitions = (yrow pair: 2) x (out-ch: 64); free = (z:16, x:32)
                stv = st.rearrange("p (z x) -> p z x", z=16)
                for yb in (0, 1):
                    dest = out.ap()[:, :, y0 + yb, :]
                    nc.sync.dma_start(dest, stv[64 * yb:64 * (yb + 1)])
    return nc


def _get_prog():
    global _PROG
    if _PROG is None:
        _PROG = _build_program()
    return _PROG


# ---------------------------------------------------------------------------
# Public entry point.
# ---------------------------------------------------------------------------

LAST_RESULTS = None


def kernel(x, w_lin0, w_lin1, tp_weight):
    global LAST_RESULTS
    from concourse.bass_utils import run_bass_kernel_spmd

    x = np.ascontiguousarray(np.asarray(x, dtype=np.float32))
    keff = _build_keff(np.asarray(w_lin0, np.float32),
                       np.asarray(w_lin1, np.float32),
                       np.asarray(tp_weight, np.float32))
    W = _build_lhst(keff)  # (75, 128, 128)
    wts = np.ascontiguousarray(W.transpose(1, 0, 2).reshape(128, NW * 128))

    # Host-side halo padding. SBUF copy1 rows (partitions 0-63) hold
    # y-coords -2..33; copy2 rows (64-127) hold y-coords -1..34.
    xpad = np.pad(x, ((0, 0), (0, 0), (2, 2), (2, 3), (2, 2)))
    in_maps = []
    shard_meta = []
    for b in range(4):
        for z0 in (0, 16):
            copy1 = xpad[b, :, z0:z0 + ZT, 0:36, :].reshape(64, -1)
            copy2 = xpad[b, :, z0:z0 + ZT, 1:37, :].reshape(64, -1)
            xs = np.ascontiguousarray(np.concatenate([copy1, copy2], 0))
            in_maps.append({"xs": xs, "wt": wts})
            shard_meta.append((b, z0))

    nc = _get_prog()
    res = run_bass_kernel_spmd(nc, in_maps, core_ids=list(range(8)))
    LAST_RESULTS = res

    out = np.empty((4, 64, 32, 32, 32), np.float32)
    for (b, z0), r in zip(shard_meta, res.results):
        out[b, :, z0:z0 + 16] = r["out"]
    return out


# revision 8
# speedup vs baseline: 17085.7323x; 17085.7323x over previous
"""Trainium2 Bass kernel for nn_Convolution_49125835932146.

The whole module (kernel synthesis + self-interaction + conv3d) reduces to a
single 64->64 channel 5x5x5 conv3d: the per-voxel self-interaction linear is
folded into the center tap of the (host-synthesized, tiny) conv kernel.

Distribution: 8 shards = batch(4) x z-half(2), one per NeuronCore. Each core
computes a (64, 16, 32, 32) output shard.

Per-core compute: PSUM tile = 128 partitions (64 out-ch x 2 adjacent y rows)
x 512 free (16 z x 32 x). Contraction K=128 = 64 in-ch x 2 adjacent y input
slices (SBUF holds x and x-shifted-by-one-y-row in partitions 0-63 / 64-127).
75 accumulating float32r matmuls per y-row-pair (5 dz x 5 dx x 3 slice-pairs)
cover all 125 taps; 10 of every 12 weight blocks are useful (83% PE util).
"""

import math

import numpy as np

# ---------------------------------------------------------------------------
# Host-side weight synthesis (mirrors the reference's kernel-synthesis stage).
# ---------------------------------------------------------------------------

SIZE = 5
MUL = 16


def _real_sh(v):
    x = v[..., 0]; y = v[..., 1]; z = v[..., 2]
    r2 = x * x + y * y + z * z
    s3 = math.sqrt(3.0); s5 = math.sqrt(5.0); s15 = math.sqrt(15.0)
    s7 = math.sqrt(7.0); s105 = math.sqrt(105.0)
    c21 = 0.5 * math.sqrt(10.5)
    c35 = 0.5 * math.sqrt(17.5)
    out = [
        np.ones_like(x),
        s3 * y, s3 * z, s3 * x,
        s15 * x * y, s15 * y * z, 0.5 * s5 * (3 * z * z - r2),
        s15 * x * z, 0.5 * s15 * (x * x - y * y),
        c35 * y * (3 * x * x - y * y), s105 * x * y * z,
        c21 * y * (5 * z * z - r2), 0.5 * s7 * z * (5 * z * z - 3 * r2),
        c21 * x * (5 * z * z - r2), 0.5 * s105 * z * (x * x - y * y),
        c35 * x * (x * x - 3 * y * y),
    ]
    return np.stack(out, axis=-1)


def _wigner3j(l1, l2, l3, rng):
    dims = (2 * l1 + 1, 2 * l2 + 1, 2 * l3 + 1)
    n = dims[0] * dims[1] * dims[2]
    rows = []
    for _ in range(4):
        A = rng.standard_normal((3, 3))
        Q, _ = np.linalg.qr(A)
        if np.linalg.det(Q) < 0:
            Q = -Q
        X = rng.standard_normal((64, 3))
        X /= np.linalg.norm(X, axis=1, keepdims=True)
        YA = _real_sh(X)
        YB = _real_sh(X @ Q.T)
        Ds = []
        for l in (l1, l2, l3):
            sl = slice(l * l, (l + 1) * (l + 1))
            D = np.linalg.lstsq(YA[:, sl], YB[:, sl], rcond=None)[0].T
            Ds.append(D)
        rows.append(np.kron(Ds[0], np.kron(Ds[1], Ds[2])) - np.eye(n))
    _, _, Vt = np.linalg.svd(np.concatenate(rows, 0))
    w = Vt[-1]
    w = w * np.sign(w[np.argmax(np.abs(w))])
    w = w / np.linalg.norm(w)
    return w.reshape(dims).astype(np.float32)


_W3J = None


def _w3j():
    global _W3J
    if _W3J is None:
        rng = np.random.default_rng(7)
        _W3J = {k: _wigner3j(*k, rng)
                for k in [(0, 0, 0), (0, 1, 1), (1, 0, 1), (1, 1, 0), (1, 2, 1)]}
    return _W3J


def _build_keff(w_lin0, w_lin1, tp_weight):
    """(64,64,5,5,5) conv kernel with the self-interaction folded into the
    center tap and the 0.1 conv scale applied: out = conv3d(x, keff, pad=2)."""
    size, mul = SIZE, MUL
    W3J = _w3j()
    r = np.linspace(-1.0, 1.0, size)
    lat = np.stack(np.meshgrid(r, r, r, indexing="ij"), axis=-1)
    d = np.sqrt(np.sum(lat * lat, axis=-1))
    v = lat / np.maximum(d, 1e-12)[..., None]
    sh = _real_sh(v)
    centers = np.linspace(0.0, 1.0, size)
    sigma = 1.0 / (size - 1)
    emb = np.exp(-np.square((d[..., None] - centers) / sigma)) / 1.12
    w = (emb @ tp_weight.astype(np.float64)) * np.cos(np.pi * d)[..., None] / size ** 1.5
    W = w.reshape(size, size, size, 5, mul, mul)
    f0 = math.sqrt(1.0 / (2 * mul))
    f1 = math.sqrt(3.0 / (3 * mul))

    def blk(C, ysl, Wp, f):
        t = f * np.einsum("abc,xyzb,xyzuw->xyzuawc", C, ysl, Wp)
        s = t.shape
        return t.reshape(size, size, size, s[3] * s[4], s[5] * s[6])

    y0 = sh[..., 0:1]; y1 = sh[..., 1:4]; y2 = sh[..., 4:9]
    b00 = blk(W3J[(0, 0, 0)], y0, W[..., 0, :, :], f0)
    b01 = blk(W3J[(0, 1, 1)], y1, W[..., 1, :, :], f1)
    b11a = blk(W3J[(1, 0, 1)], y0, W[..., 2, :, :], f1)
    b10 = blk(W3J[(1, 1, 0)], y1, W[..., 3, :, :], f0)
    b11b = blk(W3J[(1, 2, 1)], y2, W[..., 4, :, :], f1)
    k = np.concatenate([
        np.concatenate([b00, b01], axis=-1),
        np.concatenate([b10, b11a + b11b], axis=-1)], axis=-2)
    kernel = np.transpose(k, (4, 3, 0, 1, 2))  # oixyz

    inv = 1.0 / math.sqrt(mul)
    L = np.zeros((64, 64), np.float64)
    L[:16, :16] = inv * w_lin0.T
    L[16:, 16:] = inv * np.kron(w_lin1.T, np.eye(3))
    keff = 0.1 * kernel
    keff[:, :, 2, 2, 2] += L
    return keff.astype(np.float32)


def _build_lhst(keff):
    """(75, 128, 128) stationary matmul operands.
    m = (dz*5+dx)*3 + ui with u = 2*ui; W[m, i+64a, o+64b] = keff[o,i,dz,u+a-b,dx]."""
    W = np.zeros((75, 128, 128), np.float32)
    for dz in range(5):
        for dx in range(5):
            for ui, u in enumerate((0, 2, 4)):
                m = (dz * 5 + dx) * 3 + ui
                for a in (0, 1):
                    for b in (0, 1):
                        dy = u + a - b
                        if 0 <= dy < 5:
                            W[m, 64 * a:64 * a + 64, 64 * b:64 * b + 64] = \
                                keff[:, :, dz, dy, dx].T
    return W


# ---------------------------------------------------------------------------
# BIR post-processing: this container's walrus accepts at most ONE sync-wait
# per instruction; Tile emits multi-wait joins. Move extra waits onto
# injected same-engine NoOps placed immediately before the instruction.
# ---------------------------------------------------------------------------

def _split_multi_waits(bir_json: bytes) -> bytes:
    import orjson
    d = orjson.loads(bir_json)
    counter = [0]

    def fix_blocks(blocks):
        for b in blocks:
            insts = b.get("instructions", [])
            out = []
            for i in insts:
                si = i.get("sync_info")
                waits = (si or {}).get("on_wait") or []
                if len(waits) > 1:
                    for w in waits[:-1]:
                        counter[0] += 1
                        out.append({
                            "name": f"I-waitsplit-{counter[0]}",
                            "engine": i["engine"],
                            "opcode": "NoOp",
                            "ins": [],
                            "outs": [],
                            "debug": i.get("debug"),
                            "sync_info": {"on_wait": [w], "on_update": []},
                        })
                    si["on_wait"] = [waits[-1]]
                out.append(i)
            b["instructions"] = out
            fix_blocks(b.get("blocks", []))

    for fn in d["functions"]:
        fix_blocks(fn["blocks"])
    return orjson.dumps(d)


def _install_waitsplit():
    import concourse.bass_utils as bass_utils
    import concourse.bass2jax as bass2jax

    if getattr(bass_utils, "_waitsplit_installed", False):
        return
    orig = bass_utils.compile_bir_kernel

    def patched(bir_json, tmpdir, neff_name="file.neff"):
        return orig(_split_multi_waits(bytes(bir_json)), tmpdir, neff_name)

    bass_utils.compile_bir_kernel = patched
    bass_utils._waitsplit_installed = True
    if getattr(bass2jax, "compile_bir_kernel", None) is orig:
        bass2jax.compile_bir_kernel = patched


# ---------------------------------------------------------------------------
# Bass program (one per-core SPMD program, built once and cached).
# ---------------------------------------------------------------------------

_PROG = None

# SBUF x geometry: 20 z-planes, 36 y-slots, 36 x-cols (with halo), split into
# NYC y-chunks of YC slots each so compute can start before the whole x load
# lands (each matmul reads exactly one y-slot -> one chunk).
ZT, XT = 20, 36
YC, NYC = 6, 6
CHUNK = ZT * YC * XT  # 4320 per partition per chunk
NVOX = NYC * CHUNK
NW = 75


def _build_program(reps=1):
    import concourse.bass as bass
    import concourse.mybir as mybir
    import concourse.tile as tile

    _install_waitsplit()

    nc = bass.Bass("TRN2", target_bir_lowering=False, debug=False)
    xs_in = nc.dram_tensor("xs", (128, NYC, CHUNK), mybir.dt.float32r, kind="ExternalInput")
    wt_in = nc.dram_tensor("wt", (128, NW * 128), mybir.dt.float32r, kind="ExternalInput")
    out = nc.dram_tensor("out", (64, 16, 32, 32), mybir.dt.float32, kind="ExternalOutput")

    with tile.TileContext(nc) as tc:
        with tc.tile_pool(name="xpool", bufs=1) as xpool, \
             tc.tile_pool(name="wpool", bufs=1) as wpool, \
             tc.tile_pool(name="stage", bufs=4) as stpool, \
             tc.tile_pool(name="psum", bufs=4, space="PSUM") as pspool:
            # Weights split by dz (15 matrices each) so pair 0's first matmuls
            # start after ~1MB of weights + the first x chunk, not the full load.
            wtd = [wpool.tile([128, 15, 128], mybir.dt.float32r, tag=f"wt{dz}",
                              name=f"wt{dz}")
                   for dz in range(5)]
            xc = [xpool.tile([128, ZT, YC, XT], mybir.dt.float32r, tag=f"xc{c}",
                             name=f"xc{c}")
                  for c in range(NYC)]
            wt_ap = wt_in.ap().rearrange("p (d m c) -> p d m c", d=5, m=15)
            nc.sync.dma_start(wtd[0][:], wt_ap[:, 0])
            nc.sync.dma_start(
                xc[0][:],
                xs_in.ap()[:, 0, :].rearrange("p (z y x) -> p z y x", z=ZT, y=YC))
            for dz in range(1, 5):
                nc.sync.dma_start(wtd[dz][:], wt_ap[:, dz])
            for c in range(1, NYC):
                nc.sync.dma_start(
                    xc[c][:],
                    xs_in.ap()[:, c, :].rearrange("p (z y x) -> p z y x", z=ZT, y=YC))

            for _ in range(reps):
                for yp in range(16):
                    y0 = 2 * yp
                    ps = pspool.tile([128, 512], mybir.dt.float32)
                    first = True
                    for dz in range(5):
                        for dx in range(5):
                            for ui, u in enumerate((0, 2, 4)):
                                s = y0 + u
                                rhs = xc[s // YC][:, dz:dz + 16, s % YC, dx:dx + 32]
                                last = (dz == 4 and dx == 4 and ui == 2)
                                nc.tensor.matmul(ps[:], wtd[dz][:, dx * 3 + ui, :],
                                                 rhs, start=first, stop=last)
                                first = False
                    st = stpool.tile([128, 512], mybir.dt.float32)
                    nc.vector.tensor_copy(st[:], ps[:])
                    # psum partitions = (yrow pair: 2) x (out-ch: 64); free = (z:16, x:32)
                    stv = st.rearrange("p (z x) -> p z x", z=16)
                    for yb in (0, 1):
                        dest = out.ap()[:, :, y0 + yb, :]
                        nc.sync.dma_start(dest, stv[64 * yb:64 * (yb + 1)])
    return nc


def _get_prog():
    global _PROG
    if _PROG is None:
        _PROG = _build_program()
    return _PROG


# ---------------------------------------------------------------------------
# Public entry point.
# ---------------------------------------------------------------------------

LAST_RESULTS = None


def kernel(x, w_lin0, w_lin1, tp_weight):
    global LAST_RESULTS
    from concourse.bass_utils import run_bass_kernel_spmd

    x = np.ascontiguousarray(np.asarray(x, dtype=np.float32))
    keff = _build_keff(np.asarray(w_lin0, np.float32),
                       np.asarray(w_lin1, np.float32),
                       np.asarray(tp_weight, np.float32))
    W = _build_lhst(keff)  # (75, 128, 128)
    wts = np.ascontiguousarray(W.transpose(1, 0, 2).reshape(128, NW * 128))

    # Host-side halo padding. SBUF copy1 rows (partitions 0-63) hold
    # y-coords -2..33; copy2 rows (64-127) hold y-coords -1..34; y is split
    # into NYC chunks of YC slots, each chunk contiguous in DRAM.
    xpad = np.pad(x, ((0, 0), (0, 0), (2, 2), (2, 3), (2, 2)))
    in_maps = []
    shard_meta = []
    for b in range(4):
        for z0 in (0, 16):
            xs = np.empty((128, NYC, CHUNK), np.float32)
            for c in range(NYC):
                y1 = YC * c
                xs[:64, c] = xpad[b, :, z0:z0 + ZT, y1:y1 + YC, :].reshape(64, -1)
                xs[64:, c] = xpad[b, :, z0:z0 + ZT, y1 + 1:y1 + YC + 1, :].reshape(64, -1)
            in_maps.append({"xs": xs, "wt": wts})
            shard_meta.append((b, z0))

    nc = _get_prog()
    res = run_bass_kernel_spmd(nc, in_maps, core_ids=list(range(8)))
    LAST_RESULTS = res

    out = np.empty((4, 64, 32, 32, 32), np.float32)
    for (b, z0), r in zip(shard_meta, res.results):
        out[b, :, z0:z0 + 16] = r["out"]
    return out
